# revision 30
# baseline (speedup 1.0000x reference)
"""Trainium2 Bass kernel for DepthSeparableConv2d (dw3x3 + BN + ReLU + prune,
pw1x1 + BN + ReLU + prune) on (64, 512, 28, 28) fp32.

Strategy: data-parallel over batch across 8 NeuronCores (8 images/core).
Per core, channels live on SBUF partitions (4 blocks of 128):
  - depthwise 3x3 runs on the TensorEngine as per-channel diagonal matmuls
    accumulated in PSUM. Operands are fp8e4 (prune margins are ~70%, so
    fp8 cannot flip any prune decision); taps (ky=0,ky=1) of each kx are
    fused into one DoubleRow matmul (2 MACs/cycle) via an overlapping
    access pattern on a zero-padded [30,32] staging tile; ky=2 taps run
    as regular fp8 matmuls. Zero pad rows/cols implement the conv padding.
  - BN is folded into conv scale/bias on the host (inference constants).
  - the per-(batch,channel) magnitude prune is computed as reduce_max
    over the raw PSUM (max is monotone under +bias/ReLU) and applied as
    a per-partition scale inside the ScalarEngine bias+ReLU epilogue:
    out = Relu(psum*mask + bias*mask). The epilogue writes h directly in
    fp8 with contraction-pair slot interleaving for the pointwise.
  - pointwise 1x1 is a dense 512x512 matmul over pixels, also fp8
    DoubleRow (k-blocks paired), fp32 PSUM accumulation, fp32 epilogue.
"""

import os
import sys

import ml_dtypes
import numpy as np

for _p in ("/opt/trn_rl_repo",):
    if os.path.isdir(_p) and _p not in sys.path:
        sys.path.insert(0, _p)

N_CORES = 8
B_FULL = 64
B_CORE = B_FULL // N_CORES  # 8
C = 512
CB = C // 128  # 4 channel blocks
H = W = 28
HP = H + 2  # padded rows (one zero row top/bottom)
WP = 32  # padded row stride: 1 zero col left, cols 29..31 zero
HALF = 14  # psum bank split: 14*28*4B = 1568B <= 2KB bank
NH = HALF * W  # 392
EPS = 1e-5
DW_THRESH = 4.0
PW_THRESH = 0.001

_PROG = None


def _build_program():
    import concourse.bass as bass
    import concourse.bacc as bacc
    import concourse.tile as tile
    from concourse import mybir

    f32 = mybir.dt.float32
    f8 = mybir.dt.float8e4
    AX = mybir.AxisListType
    ALU = mybir.AluOpType
    ACTF = mybir.ActivationFunctionType
    DR = mybir.MatmulPerfMode.DoubleRow

    nc = bacc.Bacc()
    x_d = nc.declare_dram_parameter("x", [B_CORE, C, H, W], f32, isOutput=False)
    dwp_d = nc.declare_dram_parameter("dwp", [CB, 128, 3, 2, 128], f8, isOutput=False)
    dws_d = nc.declare_dram_parameter("dws", [CB, 128, 3, 128], f8, isOutput=False)
    pwp_d = nc.declare_dram_parameter("pwp", [2, 128, 2, C], f8, isOutput=False)
    b1_d = nc.declare_dram_parameter("b1", [128, CB], f32, isOutput=False)
    b2_d = nc.declare_dram_parameter("b2", [128, CB], f32, isOutput=False)
    out_d = nc.declare_dram_parameter("out", [B_CORE, C, H, W], f32, isOutput=True)

    with tile.TileContext(nc) as tc:
        with (
            tc.tile_pool(name="consts", bufs=1) as consts,
            tc.tile_pool(name="xp", bufs=8) as xp,
            tc.tile_pool(name="hp", bufs=6) as hp,
            tc.tile_pool(name="op", bufs=6) as op,
            tc.tile_pool(name="small", bufs=6) as small,
            tc.tile_pool(name="psp", bufs=4, space="PSUM") as psp,
        ):
            # persistent zero-padded fp8 staging tiles (2 per channel block,
            # ping-ponged over batches): pads are zeroed exactly once; the
            # f32->fp8 cast (DVE/ACT alternating) writes the interior.
            # Parity-0 tiles are zeroed first (DVE, idle at startup) so the
            # first batch can stage immediately; parity-1 follows on GpSimd.
            xq_tiles = []
            for cb in range(CB):
                pair = []
                for par in range(2):
                    xq = consts.tile(
                        [128, HP + 1, WP], f8,
                        name=f"xq_{cb}_{par}", tag=f"xq_{cb}_{par}"
                    )
                    pair.append(xq)
                xq_tiles.append(pair)
            dwp_sb = []
            dws_sb = []
            for cb in range(CB):
                pt = consts.tile([128, 3, 2, 128], f8, name=f"dwp{cb}")
                nc.scalar.dma_start(out=pt, in_=dwp_d[cb])
                dwp_sb.append(pt)
                st = consts.tile([128, 3, 128], f8, name=f"dws{cb}")
                nc.scalar.dma_start(out=st, in_=dws_d[cb])
                dws_sb.append(st)
            pwp_sb = []
            for p in range(2):
                wt = consts.tile([128, 2, C], f8, name=f"pwp{p}")
                nc.scalar.dma_start(out=wt, in_=pwp_d[p])
                pwp_sb.append(wt)
            b1_sb = consts.tile([128, CB], f32, name="b1sb")
            nc.scalar.dma_start(out=b1_sb, in_=b1_d[:, :])
            b2_sb = consts.tile([128, CB], f32, name="b2sb")
            nc.scalar.dma_start(out=b2_sb, in_=b2_d[:, :])

            def epilogue(ps_view, axis, bias_col, thresh, dest):
                """prune mask + fused bias/ReLU/mask epilogue: psum -> sbuf."""
                mx = small.tile([128, 1], f32, tag="mx", name="mx")
                nc.vector.tensor_reduce(
                    out=mx, in_=ps_view, axis=axis, op=ALU.max
                )
                # mask = ((max_psum + bias) >= thresh) — equivalent to
                # relu(max_psum + bias) >= thresh since thresh > 0
                mask = small.tile([128, 1], f32, tag="mask", name="mask")
                nc.vector.tensor_scalar(
                    out=mask, in0=mx, scalar1=bias_col, scalar2=float(thresh),
                    op0=ALU.add, op1=ALU.is_ge,
                )
                mb = small.tile([128, 1], f32, tag="mb", name="mb")
                nc.vector.tensor_scalar_mul(mb, mask, bias_col)
                nc.scalar.activation(
                    out=dest,
                    in_=ps_view,
                    func=ACTF.Relu,
                    bias=mb,
                    scale=mask,
                )

            def stage_batch(b, first=False):
                """DMA x for batch b and cast f32->fp8 into the padded tiles."""
                for cb in range(CB):
                    x_t = xp.tile([128, H, W], f32, tag="x", name=f"x_{b}_{cb}")
                    nc.sync.dma_start(
                        out=x_t, in_=x_d[b, cb * 128 : (cb + 1) * 128]
                    )
                    xq = xq_tiles[cb][b % 2]
                    if first:
                        eng = nc.vector if cb % 2 == 0 else nc.gpsimd
                        eng.memset(xq.rearrange("p y x -> p (y x)"), 0.0)
                    if (b * CB + cb) % 2 == 0:
                        nc.vector.tensor_copy(
                            out=xq[:, 1 : H + 1, 1 : W + 1], in_=x_t
                        )
                    else:
                        nc.scalar.copy(out=xq[:, 1 : H + 1, 1 : W + 1], in_=x_t)

            stage_batch(0, first=True)
            for cb in range(CB):
                nc.gpsimd.memset(
                    xq_tiles[cb][1].rearrange("p y x -> p (y x)"), 0.0
                )
            # PE warmup: harmless matmuls on the zeroed staging tile warm the
            # HAM clock gate while batch 0 finishes staging
            ps_w = psp.tile([128, 2, 512], f32, tag="ps", name="ps_w")
            for i in range(6):
                nc.tensor.matmul(
                    out=ps_w[:, i % 2, 0 : HALF * WP],
                    lhsT=dwp_sb[0][:, 0, :, :],
                    rhs=bass.AP(
                        tensor=xq_tiles[0][0].tensor,
                        offset=xq_tiles[0][0].offset,
                        ap=[xq_tiles[0][0].ap[0], [WP, 2], [1, HALF * WP]],
                    ),
                    start=(i < 2),
                    stop=(i >= 4),
                    perf_mode=DR,
                )
            stage_batch(1)

            def dw_tile(b, cb, h_pairs):
                xq = xq_tiles[cb][b % 2]
                ps1 = psp.tile([128, 2, 512], f32, tag="ps", name="ps1")
                NW = HALF * WP  # 448 columns per half (rows incl. pad cols)
                # paired taps (ky=0, ky=1) per kx: DoubleRow, slot = +1 row;
                # rhs streams full 32-wide rows contiguously — columns
                # >= 28 of each row compute garbage that the epilogue skips
                for kx in range(3):
                    for hi, hy0 in enumerate((0, HALF)):
                        rhs = bass.AP(
                            tensor=xq.tensor,
                            offset=xq.offset + hy0 * WP + kx,
                            ap=[
                                xq.ap[0],
                                [WP, 2],
                                [1, NW],
                            ],
                        )
                        nc.tensor.matmul(
                            out=ps1[:, hi, 0:NW],
                            lhsT=dwp_sb[cb][:, kx, :, :],
                            rhs=rhs,
                            start=(kx == 0),
                            stop=False,
                            perf_mode=DR,
                        )
                # single taps (ky=2) per kx: regular fp8 matmuls
                for kx in range(3):
                    for hi, hy0 in enumerate((0, HALF)):
                        rhs = bass.AP(
                            tensor=xq.tensor,
                            offset=xq.offset + (hy0 + 2) * WP + kx,
                            ap=[xq.ap[0], [1, NW]],
                        )
                        nc.tensor.matmul(
                            out=ps1[:, hi, 0:NW],
                            lhsT=dws_sb[cb][:, kx, :],
                            rhs=rhs,
                            start=False,
                            stop=(kx == 2),
                        )
                dest = h_pairs[cb // 2][:, :, cb % 2, 0:NH].rearrange(
                    "p h (y x) -> p h y x", x=W
                )
                ps1v = ps1.rearrange("p h (y x) -> p h y x", x=WP)[
                    :, :, 0:HALF, 0:W
                ]
                epilogue(ps1v, AX.XYZ, b1_sb[:, cb : cb + 1], DW_THRESH, dest)

            def pw_tile(b, m, h_pairs):
                ps2 = psp.tile([128, 2, 512], f32, tag="ps", name="ps2")
                for p in range(2):
                    for hi in range(2):
                        nc.tensor.matmul(
                            out=ps2[:, hi, 0:NH],
                            lhsT=pwp_sb[p][:, :, m * 128 : (m + 1) * 128],
                            rhs=h_pairs[p][:, hi, :, 0:NH],
                            start=(p == 0),
                            stop=(p == 1),
                            perf_mode=DR,
                        )
                o_t = op.tile([128, H * W], f32, tag="o", name=f"o_{b}_{m}")
                epilogue(
                    ps2[:, :, 0:NH],
                    AX.XY,
                    b2_sb[:, m : m + 1],
                    PW_THRESH,
                    o_t.rearrange("p (h n) -> p h n", h=2),
                )
                out_eng = nc.sync if m % 2 == 0 else nc.scalar
                out_eng.dma_start(
                    out=out_d[b, m * 128 : (m + 1) * 128].rearrange(
                        "c y x -> c (y x)"
                    ),
                    in_=o_t,
                )

            # software pipeline: DW tiles of batch b interleave with PW tiles
            # of batch b-1 so the PE never waits on the epilogue chain
            h_by_batch = {}
            for b in range(B_CORE + 1):
                if b < B_CORE:
                    h_by_batch[b] = [
                        hp.tile([128, 2, 2, 512], f8, tag="h", name=f"h_{b}_{p}")
                        for p in range(2)
                    ]
                for cb in range(CB):
                    if b < B_CORE:
                        dw_tile(b, cb, h_by_batch[b])
                    if b > 0:
                        pw_tile(b - 1, cb, h_by_batch[b - 1])
                    if cb == 1 and 1 <= b and b + 1 < B_CORE:
                        stage_batch(b + 1)
                if b > 0:
                    del h_by_batch[b - 1]

    nc.finalize()
    return nc


def _get_program():
    global _PROG
    if _PROG is None:
        _PROG = _build_program()
    return _PROG


def _prepare_inputs(inputs):
    f32 = np.float32
    f8 = ml_dtypes.float8_e4m3
    x = np.ascontiguousarray(inputs["x"], dtype=f32)
    dw_w = np.asarray(inputs["dw_w"], dtype=f32).reshape(C, 9)
    dw_b = np.asarray(inputs["dw_b"], dtype=f32)
    bn1_g = np.asarray(inputs["bn1_g"], dtype=f32)
    bn1_b = np.asarray(inputs["bn1_b"], dtype=f32)
    bn1_m = np.asarray(inputs["bn1_m"], dtype=f32)
    bn1_v = np.asarray(inputs["bn1_v"], dtype=f32)
    pw_w = np.asarray(inputs["pw_w"], dtype=f32).reshape(C, C)
    pw_b = np.asarray(inputs["pw_b"], dtype=f32)
    bn2_g = np.asarray(inputs["bn2_g"], dtype=f32)
    bn2_b = np.asarray(inputs["bn2_b"], dtype=f32)
    bn2_m = np.asarray(inputs["bn2_m"], dtype=f32)
    bn2_v = np.asarray(inputs["bn2_v"], dtype=f32)

    inv1 = (bn1_g / np.sqrt(bn1_v + f32(EPS))).astype(f32)
    inv2 = (bn2_g / np.sqrt(bn2_v + f32(EPS))).astype(f32)
    wdw = (dw_w * inv1[:, None]).astype(f8)  # [C, 9] fp8
    bias1 = (dw_b * inv1 + bn1_b - bn1_m * inv1).astype(f32)
    wpw = (pw_w * inv2[:, None]).T.astype(f8)  # [ci, co] fp8
    bias2 = (pw_b * inv2 + bn2_b - bn2_m * inv2).astype(f32)

    idx = np.arange(128)
    # dwp[cb, ci, kx, slot, co]: slot s = tap (ky=s, kx), diag over channels
    dwp = np.zeros((CB, 128, 3, 2, 128), dtype=f8)
    wr = np.asarray(wdw).reshape(CB, 128, 3, 3)  # [cb, ci, ky, kx]
    for s in range(2):
        for kx in range(3):
            dwp[:, idx, kx, s, idx] = wr[:, :, s, kx]
    # dws[cb, ci, kx, co]: tap (ky=2, kx)
    dws = np.zeros((CB, 128, 3, 128), dtype=f8)
    for kx in range(3):
        dws[:, idx, kx, idx] = wr[:, :, 2, kx]
    # pwp[p, ci, slot, co] = W'[(2p+s)*128+ci, co]
    pwp = np.zeros((2, 128, 2, C), dtype=f8)
    for p in range(2):
        for s in range(2):
            pwp[p, :, s, :] = wpw[(2 * p + s) * 128 : (2 * p + s + 1) * 128, :]

    b1_host = np.ascontiguousarray(bias1.reshape(CB, 128).T, dtype=f32)
    b2_host = np.ascontiguousarray(bias2.reshape(CB, 128).T, dtype=f32)

    in_maps = []
    for i in range(N_CORES):
        in_maps.append(
            {
                "x": x[i * B_CORE : (i + 1) * B_CORE],
                "dwp": dwp,
                "dws": dws,
                "pwp": pwp,
                "b1": b1_host,
                "b2": b2_host,
            }
        )
    return in_maps


def _run(inputs, trace=False):
    """Returns (full_output, BassKernelResults)."""
    from concourse.bass_utils import run_bass_kernel_spmd

    nc = _get_program()
    in_maps = _prepare_inputs(inputs)
    res = run_bass_kernel_spmd(
        nc, in_maps, core_ids=list(range(N_CORES)), trace=trace
    )
    outs = [res.results[i]["out"] for i in range(N_CORES)]
    full = np.concatenate(outs, axis=0)
    return full, res


def kernel(**inputs) -> np.ndarray:
    out, _ = _run(inputs, trace=False)
    return out


# revision 31
# speedup vs baseline: 1.0060x; 1.0060x over previous
"""Trainium2 Bass kernel for DepthSeparableConv2d (dw3x3 + BN + ReLU + prune,
pw1x1 + BN + ReLU + prune) on (64, 512, 28, 28) fp32.

Strategy: data-parallel over batch across 8 NeuronCores (8 images/core).
Per core, channels live on SBUF partitions (4 blocks of 128):
  - depthwise 3x3 runs on the TensorEngine as per-channel diagonal matmuls
    accumulated in PSUM. Operands are fp8e4 (prune margins are ~70%, so
    fp8 cannot flip any prune decision); taps (ky=0,ky=1) of each kx are
    fused into one DoubleRow matmul (2 MACs/cycle) via an overlapping
    access pattern on a zero-padded [30,32] staging tile; ky=2 taps run
    as regular fp8 matmuls. Zero pad rows/cols implement the conv padding.
  - BN is folded into conv scale/bias on the host (inference constants).
  - the per-(batch,channel) magnitude prune is computed as reduce_max
    over the raw PSUM (max is monotone under +bias/ReLU) and applied as
    a per-partition scale inside the ScalarEngine bias+ReLU epilogue:
    out = Relu(psum*mask + bias*mask). The epilogue writes h directly in
    fp8 with contraction-pair slot interleaving for the pointwise.
  - pointwise 1x1 is a dense 512x512 matmul over pixels, also fp8
    DoubleRow (k-blocks paired), fp32 PSUM accumulation, fp32 epilogue.
"""

import os
import sys

import ml_dtypes
import numpy as np

for _p in ("/opt/trn_rl_repo",):
    if os.path.isdir(_p) and _p not in sys.path:
        sys.path.insert(0, _p)

N_CORES = 8
B_FULL = 64
B_CORE = B_FULL // N_CORES  # 8
C = 512
CB = C // 128  # 4 channel blocks
H = W = 28
HP = H + 2  # padded rows (one zero row top/bottom)
WP = 32  # padded row stride: 1 zero col left, cols 29..31 zero
HALF = 14  # psum bank split: 14*28*4B = 1568B <= 2KB bank
NH = HALF * W  # 392
EPS = 1e-5
DW_THRESH = 4.0
PW_THRESH = 0.001

_PROG = None


def _build_program():
    import concourse.bass as bass
    import concourse.bacc as bacc
    import concourse.tile as tile
    from concourse import mybir

    f32 = mybir.dt.float32
    f8 = mybir.dt.float8e4
    AX = mybir.AxisListType
    ALU = mybir.AluOpType
    ACTF = mybir.ActivationFunctionType
    DR = mybir.MatmulPerfMode.DoubleRow

    nc = bacc.Bacc()
    x_d = nc.declare_dram_parameter("x", [B_CORE, C, H, W], f32, isOutput=False)
    dwp_d = nc.declare_dram_parameter("dwp", [CB, 128, 3, 2, 128], f8, isOutput=False)
    dws_d = nc.declare_dram_parameter("dws", [CB, 128, 3, 128], f8, isOutput=False)
    pwp_d = nc.declare_dram_parameter("pwp", [2, 128, 2, C], f8, isOutput=False)
    b1_d = nc.declare_dram_parameter("b1", [128, CB], f32, isOutput=False)
    b2_d = nc.declare_dram_parameter("b2", [128, CB], f32, isOutput=False)
    out_d = nc.declare_dram_parameter("out", [B_CORE, C, H, W], f32, isOutput=True)

    with tile.TileContext(nc) as tc:
        with (
            tc.tile_pool(name="consts", bufs=1) as consts,
            tc.tile_pool(name="xp", bufs=8) as xp,
            tc.tile_pool(name="hp", bufs=6) as hp,
            tc.tile_pool(name="op", bufs=6) as op,
            tc.tile_pool(name="small", bufs=6) as small,
            tc.tile_pool(name="psp", bufs=4, space="PSUM") as psp,
        ):
            # persistent zero-padded fp8 staging tiles (2 per channel block,
            # ping-ponged over batches): pads are zeroed exactly once; the
            # f32->fp8 cast (DVE/ACT alternating) writes the interior.
            # Parity-0 tiles are zeroed first (DVE, idle at startup) so the
            # first batch can stage immediately; parity-1 follows on GpSimd.
            xq_tiles = []
            for cb in range(CB):
                pair = []
                for par in range(2):
                    xq = consts.tile(
                        [128, HP + 1, WP], f8,
                        name=f"xq_{cb}_{par}", tag=f"xq_{cb}_{par}"
                    )
                    pair.append(xq)
                xq_tiles.append(pair)
            dwp_sb = []
            dws_sb = []
            for cb in range(CB):
                pt = consts.tile([128, 3, 2, 128], f8, name=f"dwp{cb}")
                nc.scalar.dma_start(out=pt, in_=dwp_d[cb])
                dwp_sb.append(pt)
                st = consts.tile([128, 3, 128], f8, name=f"dws{cb}")
                nc.scalar.dma_start(out=st, in_=dws_d[cb])
                dws_sb.append(st)
            pwp_sb = []
            for p in range(2):
                wt = consts.tile([128, 2, C], f8, name=f"pwp{p}")
                nc.scalar.dma_start(out=wt, in_=pwp_d[p])
                pwp_sb.append(wt)
            b1_sb = consts.tile([128, CB], f32, name="b1sb")
            nc.scalar.dma_start(out=b1_sb, in_=b1_d[:, :])
            b2_sb = consts.tile([128, CB], f32, name="b2sb")
            nc.scalar.dma_start(out=b2_sb, in_=b2_d[:, :])

            def epilogue(ps_view, axis, bias_col, thresh, dest):
                """prune mask + fused bias/ReLU/mask epilogue: psum -> sbuf."""
                mx = small.tile([128, 1], f32, tag="mx", name="mx")
                nc.vector.tensor_reduce(
                    out=mx, in_=ps_view, axis=axis, op=ALU.max
                )
                # mask = ((max_psum + bias) >= thresh) — equivalent to
                # relu(max_psum + bias) >= thresh since thresh > 0
                mask = small.tile([128, 1], f32, tag="mask", name="mask")
                nc.vector.tensor_scalar(
                    out=mask, in0=mx, scalar1=bias_col, scalar2=float(thresh),
                    op0=ALU.add, op1=ALU.is_ge,
                )
                mb = small.tile([128, 1], f32, tag="mb", name="mb")
                nc.vector.tensor_scalar_mul(mb, mask, bias_col)
                nc.scalar.activation(
                    out=dest,
                    in_=ps_view,
                    func=ACTF.Relu,
                    bias=mb,
                    scale=mask,
                )

            def stage_batch(b, first=False):
                """DMA x for batch b and cast f32->fp8 into the padded tiles."""
                for cb in range(CB):
                    x_t = xp.tile([128, H, W], f32, tag="x", name=f"x_{b}_{cb}")
                    nc.sync.dma_start(
                        out=x_t, in_=x_d[b, cb * 128 : (cb + 1) * 128]
                    )
                    xq = xq_tiles[cb][b % 2]
                    if first:
                        eng = nc.vector if cb % 2 == 0 else nc.gpsimd
                        eng.memset(xq.rearrange("p y x -> p (y x)"), 0.0)
                    if (b * CB + cb) % 2 == 0:
                        nc.vector.tensor_copy(
                            out=xq[:, 1 : H + 1, 1 : W + 1], in_=x_t
                        )
                    else:
                        nc.scalar.copy(out=xq[:, 1 : H + 1, 1 : W + 1], in_=x_t)

            stage_batch(0, first=True)
            for cb in range(CB):
                nc.gpsimd.memset(
                    xq_tiles[cb][1].rearrange("p y x -> p (y x)"), 0.0
                )
            # PE warmup: harmless matmuls on the zeroed staging tile warm the
            # HAM clock gate while batch 0 finishes staging
            ps_w = psp.tile([128, 2, 512], f32, tag="ps", name="ps_w")
            for i in range(6):
                nc.tensor.matmul(
                    out=ps_w[:, i % 2, 0 : HALF * WP],
                    lhsT=dwp_sb[0][:, 0, :, :],
                    rhs=bass.AP(
                        tensor=xq_tiles[0][0].tensor,
                        offset=xq_tiles[0][0].offset,
                        ap=[xq_tiles[0][0].ap[0], [WP, 2], [1, HALF * WP]],
                    ),
                    start=(i < 2),
                    stop=(i >= 4),
                    perf_mode=DR,
                )
            stage_batch(1)

            def dw_tile(b, cb, h_pairs):
                xq = xq_tiles[cb][b % 2]
                ps1 = psp.tile([128, 2, 512], f32, tag="ps", name="ps1")
                NW = HALF * WP  # 448 columns per half (rows incl. pad cols)
                # paired taps (ky=0, ky=1) per kx: DoubleRow, slot = +1 row;
                # rhs streams full 32-wide rows contiguously — columns
                # >= 28 of each row compute garbage that the epilogue skips
                for kx in range(3):
                    for hi, hy0 in enumerate((0, HALF)):
                        rhs = bass.AP(
                            tensor=xq.tensor,
                            offset=xq.offset + hy0 * WP + kx,
                            ap=[
                                xq.ap[0],
                                [WP, 2],
                                [1, NW],
                            ],
                        )
                        nc.tensor.matmul(
                            out=ps1[:, hi, 0:NW],
                            lhsT=dwp_sb[cb][:, kx, :, :],
                            rhs=rhs,
                            start=(kx == 0),
                            stop=False,
                            perf_mode=DR,
                        )
                # single taps (ky=2) per kx: regular fp8 matmuls
                for kx in range(3):
                    for hi, hy0 in enumerate((0, HALF)):
                        rhs = bass.AP(
                            tensor=xq.tensor,
                            offset=xq.offset + (hy0 + 2) * WP + kx,
                            ap=[xq.ap[0], [1, NW]],
                        )
                        nc.tensor.matmul(
                            out=ps1[:, hi, 0:NW],
                            lhsT=dws_sb[cb][:, kx, :],
                            rhs=rhs,
                            start=False,
                            stop=(kx == 2),
                        )
                dest = h_pairs[cb // 2][:, :, cb % 2, 0:NH].rearrange(
                    "p h (y x) -> p h y x", x=W
                )
                ps1v = ps1.rearrange("p h (y x) -> p h y x", x=WP)[
                    :, :, 0:HALF, 0:W
                ]
                epilogue(ps1v, AX.XYZ, b1_sb[:, cb : cb + 1], DW_THRESH, dest)

            def pw_tile(b, m, h_pairs):
                ps2 = psp.tile([128, 2, 512], f32, tag="ps", name="ps2")
                for p in range(2):
                    for hi in range(2):
                        nc.tensor.matmul(
                            out=ps2[:, hi, 0:NH],
                            lhsT=pwp_sb[p][:, :, m * 128 : (m + 1) * 128],
                            rhs=h_pairs[p][:, hi, :, 0:NH],
                            start=(p == 0),
                            stop=(p == 1),
                            perf_mode=DR,
                        )
                o_t = op.tile([128, H * W], f32, tag="o", name=f"o_{b}_{m}")
                epilogue(
                    ps2[:, :, 0:NH],
                    AX.XY,
                    b2_sb[:, m : m + 1],
                    PW_THRESH,
                    o_t.rearrange("p (h n) -> p h n", h=2),
                )
                out_eng = nc.sync if m % 2 == 1 else nc.scalar
                out_eng.dma_start(
                    out=out_d[b, m * 128 : (m + 1) * 128].rearrange(
                        "c y x -> c (y x)"
                    ),
                    in_=o_t,
                )

            # software pipeline: DW tiles of batch b interleave with PW tiles
            # of batch b-1 so the PE never waits on the epilogue chain
            h_by_batch = {}
            for b in range(B_CORE + 1):
                if b < B_CORE:
                    h_by_batch[b] = [
                        hp.tile([128, 2, 2, 512], f8, tag="h", name=f"h_{b}_{p}")
                        for p in range(2)
                    ]
                for cb in range(CB):
                    if b < B_CORE:
                        dw_tile(b, cb, h_by_batch[b])
                    if b > 0:
                        pw_tile(b - 1, cb, h_by_batch[b - 1])
                    if cb == 1 and 1 <= b and b + 1 < B_CORE:
                        stage_batch(b + 1)
                if b > 0:
                    del h_by_batch[b - 1]

    nc.finalize()
    return nc


def _get_program():
    global _PROG
    if _PROG is None:
        _PROG = _build_program()
    return _PROG


def _prepare_inputs(inputs):
    f32 = np.float32
    f8 = ml_dtypes.float8_e4m3
    x = np.ascontiguousarray(inputs["x"], dtype=f32)
    dw_w = np.asarray(inputs["dw_w"], dtype=f32).reshape(C, 9)
    dw_b = np.asarray(inputs["dw_b"], dtype=f32)
    bn1_g = np.asarray(inputs["bn1_g"], dtype=f32)
    bn1_b = np.asarray(inputs["bn1_b"], dtype=f32)
    bn1_m = np.asarray(inputs["bn1_m"], dtype=f32)
    bn1_v = np.asarray(inputs["bn1_v"], dtype=f32)
    pw_w = np.asarray(inputs["pw_w"], dtype=f32).reshape(C, C)
    pw_b = np.asarray(inputs["pw_b"], dtype=f32)
    bn2_g = np.asarray(inputs["bn2_g"], dtype=f32)
    bn2_b = np.asarray(inputs["bn2_b"], dtype=f32)
    bn2_m = np.asarray(inputs["bn2_m"], dtype=f32)
    bn2_v = np.asarray(inputs["bn2_v"], dtype=f32)

    inv1 = (bn1_g / np.sqrt(bn1_v + f32(EPS))).astype(f32)
    inv2 = (bn2_g / np.sqrt(bn2_v + f32(EPS))).astype(f32)
    wdw = (dw_w * inv1[:, None]).astype(f8)  # [C, 9] fp8
    bias1 = (dw_b * inv1 + bn1_b - bn1_m * inv1).astype(f32)
    wpw = (pw_w * inv2[:, None]).T.astype(f8)  # [ci, co] fp8
    bias2 = (pw_b * inv2 + bn2_b - bn2_m * inv2).astype(f32)

    idx = np.arange(128)
    # dwp[cb, ci, kx, slot, co]: slot s = tap (ky=s, kx), diag over channels
    dwp = np.zeros((CB, 128, 3, 2, 128), dtype=f8)
    wr = np.asarray(wdw).reshape(CB, 128, 3, 3)  # [cb, ci, ky, kx]
    for s in range(2):
        for kx in range(3):
            dwp[:, idx, kx, s, idx] = wr[:, :, s, kx]
    # dws[cb, ci, kx, co]: tap (ky=2, kx)
    dws = np.zeros((CB, 128, 3, 128), dtype=f8)
    for kx in range(3):
        dws[:, idx, kx, idx] = wr[:, :, 2, kx]
    # pwp[p, ci, slot, co] = W'[(2p+s)*128+ci, co]
    pwp = np.zeros((2, 128, 2, C), dtype=f8)
    for p in range(2):
        for s in range(2):
            pwp[p, :, s, :] = wpw[(2 * p + s) * 128 : (2 * p + s + 1) * 128, :]

    b1_host = np.ascontiguousarray(bias1.reshape(CB, 128).T, dtype=f32)
    b2_host = np.ascontiguousarray(bias2.reshape(CB, 128).T, dtype=f32)

    in_maps = []
    for i in range(N_CORES):
        in_maps.append(
            {
                "x": x[i * B_CORE : (i + 1) * B_CORE],
                "dwp": dwp,
                "dws": dws,
                "pwp": pwp,
                "b1": b1_host,
                "b2": b2_host,
            }
        )
    return in_maps


def _run(inputs, trace=False):
    """Returns (full_output, BassKernelResults)."""
    from concourse.bass_utils import run_bass_kernel_spmd

    nc = _get_program()
    in_maps = _prepare_inputs(inputs)
    res = run_bass_kernel_spmd(
        nc, in_maps, core_ids=list(range(N_CORES)), trace=trace
    )
    outs = [res.results[i]["out"] for i in range(N_CORES)]
    full = np.concatenate(outs, axis=0)
    return full, res


def kernel(**inputs) -> np.ndarray:
    out, _ = _run(inputs, trace=False)
    return out
